# revision 20
# baseline (speedup 1.0000x reference)
"""Multi-head attention (B=2, N=2048, D=1024, H=16) on 8 TRN2 NeuronCores.

Sharding: core c in 0..7 handles batch b=c//4 and head group hg=c%4 (4 heads
of 16).  Each core computes QKV for its heads, materialized attention, and a
partial projection (proj is row-split over heads); the host sums the 4
partials per batch and adds proj bias.  No device collectives.

Device layouts are feature-on-partition / tokens-on-free ("transposed"):
  xt   [1024, 2048]  x[b]^T                     bf16
  qk   [128 feats, 2048 toks] per feat-block    bf16  (QKV matmul + bias)
  vt   [128 toks, 4, 65] = [v_h | 1]            bf16  (ones col -> softmax sums)
  E^T  [128 Nk, 2*512] = exp(S^T * scale)       bf16  (ACT exp, psum->sbuf)
  AV   psum [65, 512]; row 64 = sums            fp32 accum
  out  [1024, 2048] partial (P_c @ O)^T         fp32

Schedule: the ACT engine (exp, 128 instrs x ~1.1us = ~143us/core) is the
floor; PE total (~137us at 2.4GHz) is just under it.  Everything is ordered
so exp starts as early as the DMA allows and never waits:
  - x^T is loaded chunk-0-first, split across the sync+gpsimd queues;
    weights go on the scalar/vector queues in need-order.
  - prologue emits only what unit (0,0)'s first scores need, then unit 0's
    kb loop self-interleaves its remaining scores (lag 2) while carrying
    v-groups and later k-groups as PE filler.
  - units run cc-major ((0,0),(1,0),(0,1),...); each unit's kb loop emits
    the next unit's scores (exp stream stays dense) plus paced fillers
    (q-groups, then proj groups for completed chunks).
  - softmax normalization: DVE reciprocal on the AV psum sum-row, gpsimd
    partition-broadcast (no DRAM round-trip), DVE multiply into ot.
"""

import numpy as np

B, N, DIM, H, DH = 2, 2048, 1024, 16, 64
SCALE = DH ** -0.5
NCORE = 8
HPC = 4            # heads per core
F = HPC * DH       # 256 features per core-headgroup
CH = 512           # token chunk (matmul moving free dim)
NCH = N // CH      # 4
KT = DIM // 128    # 8 k-tiles over model dim
TB = N // 128      # 16 token blocks
_cache = {}


def _build(debug=False):
    from contextlib import ExitStack

    import concourse.mybir as mybir
    from concourse import bacc
    from concourse.tile import TileContext

    f32 = mybir.dt.float32
    bf16 = mybir.dt.bfloat16
    nc = bacc.Bacc("TRN2", target_bir_lowering=False)

    xt_d = nc.declare_dram_parameter("xt", [DIM, N], bf16, isOutput=False)
    wqk_d = nc.declare_dram_parameter("wqk", [DIM, 2 * F], bf16, isOutput=False)
    wv_d = nc.declare_dram_parameter("wv", [DIM, F], bf16, isOutput=False)
    bqk_d = nc.declare_dram_parameter("bqk", [2 * F], f32, isOutput=False)
    bv_d = nc.declare_dram_parameter("bv", [F], f32, isOutput=False)
    pw_d = nc.declare_dram_parameter("pw", [F, DIM], bf16, isOutput=False)
    out_d = nc.declare_dram_parameter("out", [DIM, N], f32, isOutput=True)
    rscr = nc.dram_tensor("rscr", [2, NCH, 2 * CH], f32)
    if debug:
        dqk = nc.declare_dram_parameter("dqk", [4, NCH, 128, CH], bf16, isOutput=True)
        dvt = nc.declare_dram_parameter("dvt", [TB, 128, HPC, DH + 1], bf16, isOutput=True)
        dot = nc.declare_dram_parameter("dot", [2, NCH, 128, CH], bf16, isOutput=True)
        drec = nc.declare_dram_parameter("drec", [2, NCH, 64, 2 * CH], f32, isOutput=True)
        de = nc.declare_dram_parameter("de", [4, 128, 2 * CH], bf16, isOutput=True)

    xt_r = xt_d.ap().rearrange("(t p) n -> p t n", p=128)     # [128, 8, 2048]
    wqk_r = wqk_d.ap().rearrange("(t p) m -> p t m", p=128)   # [128, 8, 512]
    wv_r = wv_d.ap().rearrange("(t p) m -> p t m", p=128)     # [128, 8, 256]
    pw_r = pw_d.ap().rearrange("(t p) m -> p t m", p=128)     # [128, 2, 1024]
    out_r = out_d.ap().rearrange("(t p) n -> t p n", p=128)   # [8][128, 2048]

    with TileContext(nc) as tc, ExitStack() as st:
        consts = st.enter_context(tc.tile_pool(name="consts", bufs=1))
        qkp = st.enter_context(tc.tile_pool(name="qkp", bufs=1))
        vtp = st.enter_context(tc.tile_pool(name="vtp", bufs=1))
        otp = st.enter_context(tc.tile_pool(name="otp", bufs=1))
        ep = st.enter_context(tc.tile_pool(name="ep", bufs=2))
        recp = st.enter_context(tc.tile_pool(name="recp", bufs=2))
        outs = st.enter_context(tc.tile_pool(name="outs", bufs=3))
        ps_s = st.enter_context(tc.tile_pool(name="ps_s", bufs=2, space="PSUM"))
        ps_mm = st.enter_context(tc.tile_pool(name="ps_mm", bufs=2, space="PSUM"))
        ps_av = st.enter_context(tc.tile_pool(name="ps_av", bufs=2, space="PSUM"))

        # ---- DMA: weights on scalar queue (need-order, 128-feat blocks), x
        # chunk low-halves on sync, high-halves + wv + pw on gpsimd.
        bqk_sb = consts.tile([128, 4], f32)
        nc.scalar.dma_start(out=bqk_sb, in_=bqk_d.ap().rearrange("(f p) -> p f", p=128))
        bv_sb = consts.tile([128, F], f32)
        nc.scalar.dma_start(out=bv_sb, in_=bv_d.ap().partition_broadcast(128))
        # 2D per-ktile weight DMAs, k-half first (unit (0,0)'s scores need k
        # before q); few scalar-queue issues so the exps behind them start early.
        wqk_sb = consts.tile([128, KT, 2 * F], bf16, tag="wqk", name="wqk")
        for t in range(KT):
            nc.scalar.dma_start(out=wqk_sb[:, t, F:], in_=wqk_r[:, t, F:])
        for t in range(KT):
            nc.scalar.dma_start(out=wqk_sb[:, t, :F], in_=wqk_r[:, t, :F])
        wv_sb = consts.tile([128, KT, F], bf16, tag="wv", name="wv")
        pw_sb = consts.tile([128, 2, DIM], bf16, tag="pw", name="pw")
        for t in range(KT):
            nc.gpsimd.dma_start(out=wv_sb[:, t, :], in_=wv_r[:, t, :])
        for t in range(2):
            nc.gpsimd.dma_start(out=pw_sb[:, t, :], in_=pw_r[:, t, :])

        # x chunk-major 2D loads on the sync queue (HWDGE)
        xc_sb = [consts.tile([128, KT, CH], bf16, tag=f"x{ch}", name=f"x{ch}")
                 for ch in range(NCH)]
        for ch in range(NCH):
            cs = slice(ch * CH, (ch + 1) * CH)
            for t in range(KT):
                nc.sync.dma_start(out=xc_sb[ch][:, t, :], in_=xt_r[:, t, cs])

        # ---- SBUF result tiles
        # qk_sb[i][ch]: i=0,1 -> q features of head pair i; i=2,3 -> k of pair i-2
        qk_sb = [[qkp.tile([128, CH], bf16, tag=f"qk{i}_{ch}", name=f"qk{i}_{ch}")
                  for ch in range(NCH)] for i in range(4)]
        vt_sb = [vtp.tile([128, HPC, DH + 1], bf16, tag=f"vt{tb}", name=f"vt{tb}")
                 for tb in range(TB)]
        ot_sb = [[otp.tile([128, CH], bf16, tag=f"ot{t}_{cc}", name=f"ot{t}_{cc}")
                  for cc in range(NCH)] for t in range(2)]

        # ---- emitters ------------------------------------------------------
        qk_ps = {}

        def emit_qk_half(i, ch, half):
            """Half (4 ktiles) of a 128-feature QKV block; bias-add on half 1."""
            wo = i * 128
            if half == 0:
                qk_ps[(i, ch)] = ps_mm.tile([128, CH], f32, tag="mm", name=f"qk{i}_{ch}ps")
            ps = qk_ps[(i, ch)]
            for t in range(half * 4, half * 4 + 4):
                nc.tensor.matmul(
                    ps, wqk_sb[:, t, wo:wo + 128], xc_sb[ch][:, t, :],
                    start=(t == 0), stop=(t == KT - 1),
                )
            if half == 1:
                nc.vector.tensor_scalar_add(
                    out=qk_sb[i][ch], in0=ps, scalar1=bqk_sb[:, i:i + 1],
                )
                del qk_ps[(i, ch)]
                if debug:
                    nc.sync.dma_start(out=dqk.ap()[i, ch], in_=qk_sb[i][ch])

        def emit_qk(i, ch):
            emit_qk_half(i, ch, 0)
            emit_qk_half(i, ch, 1)

        def emit_v(tb):
            ps = ps_mm.tile([128, F], f32, tag="mm", name=f"v{tb}ps")
            for t in range(KT):
                nc.tensor.matmul(
                    ps,
                    xc_sb[tb // 4][:, t, (tb % 4) * 128:(tb % 4 + 1) * 128],
                    wv_sb[:, t, :],
                    start=(t == 0), stop=(t == KT - 1),
                )
            for hh in range(HPC):
                nc.vector.tensor_add(
                    out=vt_sb[tb][:, hh, :DH],
                    in0=ps[:, hh * DH:(hh + 1) * DH],
                    in1=bv_sb[:, hh * DH:(hh + 1) * DH],
                )
            nc.vector.memset(vt_sb[tb][:, :, DH:], 1.0)
            if debug:
                nc.sync.dma_start(out=dvt.ap()[tb], in_=vt_sb[tb])

        et_store = {}

        def emit_s(u, kb):
            """Scores + exp for unit u=(hp,cc), k-block kb: S^T [128k, 2*512q]."""
            hp, cc = u
            sp = ps_s.tile([128, 2 * CH], f32, tag="sp", name=f"sp{hp}_{cc}_{kb}")
            for j in range(2):
                kt = qk_sb[2 + hp][kb // 4]
                qt = qk_sb[hp][cc]
                nc.tensor.matmul(
                    sp[:, j * CH:(j + 1) * CH],
                    kt[j * 64:j * 64 + 64, (kb % 4) * 128:(kb % 4 + 1) * 128],
                    qt[j * 64:j * 64 + 64, :],
                    start=True, stop=True,
                )
            e = ep.tile([128, 2 * CH], bf16, tag=f"e{kb}", name=f"e{hp}_{cc}_{kb}")
            nc.scalar.activation(
                out=e, in_=sp, func=mybir.ActivationFunctionType.Exp, scale=SCALE,
            )
            et_store[(u, kb)] = e
            if debug and u == (0, 0) and kb < 4:
                nc.sync.dma_start(out=de.ap()[kb], in_=e)

        def emit_av(u, avs, kb):
            hp, cc = u
            e = et_store[(u, kb)] if kb < TB - 1 else et_store.pop((u, kb))
            if kb == TB - 1:
                for k2 in range(TB - 1):
                    et_store.pop((u, k2))
            for j in range(2):
                nc.tensor.matmul(
                    avs[j],
                    vt_sb[kb][:, 2 * hp + j, :],
                    e[:, j * CH:(j + 1) * CH],
                    start=(kb == 0), stop=(kb == TB - 1),
                )

        def finish_unit(u, avs, eng=None):
            """Softmax normalize: 1/sumexp (DVE), DMA-broadcast, mul -> ot."""
            hp, cc = u
            # Copy avs out of PSUM promptly (frees the ps_av ring for the next
            # unit); custom-DVE ops also mishandle PSUM partition offsets on
            # HW, so reciprocal reads the SBUF copy.
            stg = recp.tile([64, 2 * CH], f32, tag="stg", name=f"stg{hp}_{cc}")
            sums = recp.tile([1, 2 * CH], f32, tag="sums", name=f"sums{hp}_{cc}")
            for j in range(2):
                nc.vector.tensor_copy(
                    out=stg[:, j * CH:(j + 1) * CH], in_=avs[j][0:64, :]
                )
                nc.vector.tensor_copy(
                    out=sums[:, j * CH:(j + 1) * CH], in_=avs[j][64:65, :]
                )
            rec = recp.tile([1, 2 * CH], f32, tag="rec", name=f"rec{hp}_{cc}")
            nc.vector.reciprocal_approx_fast(out=rec, in_=sums)
            (eng or nc.gpsimd).dma_start(out=rscr.ap()[hp, cc], in_=rec)
            rec64 = recp.tile([64, 2 * CH], f32, tag="rec64", name=f"rb{hp}_{cc}")
            (eng or nc.gpsimd).dma_start(
                out=rec64, in_=rscr.ap()[hp, cc].partition_broadcast(64)
            )
            if debug:
                nc.sync.dma_start(out=drec.ap()[hp, cc], in_=rec64)
            for j in range(2):
                # gpsimd, not DVE: these wait on the rec64 round-trip, and the
                # in-order DVE queue must stay clear for os copies.
                nc.gpsimd.tensor_mul(
                    out=ot_sb[hp][cc][j * 64:j * 64 + 64, :],
                    in0=stg[:, j * CH:(j + 1) * CH],
                    in1=rec64[:, j * CH:(j + 1) * CH],
                )
            if debug:
                nc.sync.dma_start(out=dot.ap()[hp, cc], in_=ot_sb[hp][cc])

        def emit_proj(fb, cc, eng=None):
            ps = ps_mm.tile([128, CH], f32, tag="mm", name=f"pj{fb}_{cc}")
            for t in range(2):
                nc.tensor.matmul(
                    ps, pw_sb[:, t, fb * 128:(fb + 1) * 128], ot_sb[t][cc],
                    start=(t == 0), stop=(t == 1),
                )
            os = outs.tile([128, CH], f32, tag="os", name=f"os{fb}_{cc}")
            nc.vector.tensor_copy(out=os, in_=ps)
            (eng or nc.sync).dma_start(out=out_r[fb][:, cc * CH:(cc + 1) * CH], in_=os)

        # ---- schedule ------------------------------------------------------
        units = [(hp, cc) for cc in range(NCH) for hp in (0, 1)]  # cc-major

        # prologue: only what unit (0,0) needs to get exp flowing
        u0 = units[0]
        emit_qk(2, 0)          # k pair0 ch0
        emit_qk(0, 0)          # q pair0 ch0
        emit_s(u0, 0)
        emit_s(u0, 1)
        emit_v(0)
        emit_v(1)
        emit_qk(3, 0)          # k pair1 ch0  (unit (1,0) scores during u0 loop)
        emit_qk(1, 0)          # q pair1 ch0

        # unit-0 loop fillers: (kb -> list of closures), deadlines honored:
        #   v(tb) by kb=tb; k(2,ch) by kb=4ch-2; k(3,ch) by kb=4ch
        f0 = {
            0: [lambda: emit_v(2), lambda: emit_qk_half(2, 1, 0)],
            1: [lambda: emit_v(3), lambda: emit_qk_half(2, 1, 1)],
            2: [lambda: emit_v(4), lambda: emit_qk_half(3, 1, 0)],
            3: [lambda: emit_v(5), lambda: emit_qk_half(3, 1, 1)],
            4: [lambda: emit_v(6), lambda: emit_qk_half(2, 2, 0)],
            5: [lambda: emit_v(7), lambda: emit_qk_half(2, 2, 1)],
            6: [lambda: emit_v(8), lambda: emit_qk_half(3, 2, 0)],
            7: [lambda: emit_v(9), lambda: emit_qk_half(3, 2, 1)],
            8: [lambda: emit_v(10), lambda: emit_qk_half(2, 3, 0)],
            9: [lambda: emit_v(11), lambda: emit_qk_half(2, 3, 1)],
            10: [lambda: emit_v(12), lambda: emit_qk_half(3, 3, 0)],
            11: [lambda: emit_v(13), lambda: emit_qk_half(3, 3, 1)],
            12: [lambda: emit_v(14)],
            13: [lambda: emit_v(15)],
            14: [lambda: emit_qk_half(0, 1, 0)],
            15: [lambda: emit_qk_half(0, 1, 1)],
        }
        # later-unit fillers: q-groups spread over units 1..5, proj groups for
        # chunk cc after unit (1,cc) -> 4 per unit into units 2..7, rest in tail
        f_rest = {
            (1, 0): [lambda: emit_qk_half(1, 1, 0)],
            (1, 1): [lambda: emit_qk_half(1, 1, 1)],
            (1, 4): [lambda: emit_qk_half(0, 2, 0)],
            (1, 5): [lambda: emit_qk_half(0, 2, 1)],
            (1, 8): [lambda: emit_qk_half(1, 2, 0)],
            (1, 9): [lambda: emit_qk_half(1, 2, 1)],
            (1, 12): [lambda: emit_qk_half(0, 3, 0)],
            (1, 13): [lambda: emit_qk_half(0, 3, 1)],
            (2, 0): [lambda: emit_qk_half(1, 3, 0)],
            (2, 1): [lambda: emit_qk_half(1, 3, 1)],
        }
        for i in (2, 3):     # proj chunk 0 during units 2,3
            for s_, fb in zip((2, 6, 10, 14), range(4)):
                f_rest.setdefault((i, s_), []).append(
                    (lambda fb=fb + 4 * (i - 2): emit_proj(fb, 0))
                )
        for i in (4, 5):     # proj chunk 1
            for s_, fb in zip((2, 6, 10, 14), range(4)):
                f_rest.setdefault((i, s_), []).append(
                    (lambda fb=fb + 4 * (i - 4): emit_proj(fb, 1))
                )
        for i in (6, 7):     # proj chunk 2; unit 7's stores on the idle scalar queue
            for s_, fb in zip((2, 6, 10, 14), range(4)):
                f_rest.setdefault((i, s_), []).append(
                    (lambda fb=fb + 4 * (i - 6): emit_proj(fb, 2))
                )

        for i, u in enumerate(units):
            hp, cc = u
            avs = [
                ps_av.tile([65, CH], f32, tag="av", name=f"av{hp}_{cc}_{j}")
                for j in range(2)
            ]
            nxt = units[i + 1] if i + 1 < len(units) else None
            for kb in range(TB):
                if i == 0:
                    for fn in f0.get(kb, ()):
                        fn()
                    if kb + 2 < TB:
                        emit_s(u, kb + 2)
                else:
                    for fn in f_rest.get((i, kb), ()):
                        fn()
                if nxt is not None:
                    emit_s(nxt, kb)
                emit_av(u, avs, kb)
            finish_unit(u, avs)

        for fb in range(KT):
            emit_proj(fb, 3)

    nc.finalize()
    return nc


def _in_maps(x, qkv_w, qkv_b, proj_w):
    import ml_dtypes

    bf = ml_dtypes.bfloat16
    maps = []
    for c in range(NCORE):
        b, hg = c // 4, c % 4
        fs = slice(hg * F, (hg + 1) * F)
        wqk = np.concatenate([qkv_w[fs], qkv_w[DIM:][fs]], 0)        # [512,1024]
        bqk = np.concatenate([qkv_b[fs], qkv_b[DIM:][fs]], 0)
        maps.append({
            "xt": np.ascontiguousarray(x[b].T).astype(bf),
            "wqk": np.ascontiguousarray(wqk.T).astype(bf),
            "wv": np.ascontiguousarray(qkv_w[2 * DIM:][fs].T).astype(bf),
            "bqk": np.ascontiguousarray(bqk),
            "bv": np.ascontiguousarray(qkv_b[2 * DIM:][fs]),
            "pw": np.ascontiguousarray(proj_w[:, fs].T).astype(bf),
        })
    return maps


def _run(inputs, trace=False, trace_kwargs=None):
    from concourse.bass_utils import run_bass_kernel_spmd

    if "nc" not in _cache:
        _cache["nc"] = _build()
    nc = _cache["nc"]
    maps = _in_maps(inputs["x"], inputs["qkv_w"], inputs["qkv_b"], inputs["proj_w"])
    res = run_bass_kernel_spmd(
        nc, maps, list(range(NCORE)), trace=trace, **(trace_kwargs or {})
    )
    outs = [r["out"] for r in res.results]              # [1024, 2048] partials
    full = np.empty((B, N, DIM), dtype=np.float32)
    for b in range(B):
        acc = np.array(outs[4 * b], dtype=np.float32)
        for c in range(4 * b + 1, 4 * b + 4):
            acc += np.asarray(outs[c], dtype=np.float32)
        full[b] = acc.T + inputs["proj_b"]
    return full, res


def kernel(**inputs) -> np.ndarray:
    out, _ = _run(inputs, trace=False)
    return out


# revision 24
# speedup vs baseline: 1.0040x; 1.0040x over previous
"""Multi-head attention (B=2, N=2048, D=1024, H=16) on 8 TRN2 NeuronCores.

Sharding: core c in 0..7 handles batch b=c//4 and head group hg=c%4 (4 heads
of 16).  Each core computes QKV for its heads, materialized attention, and a
partial projection (proj is row-split over heads); the host sums the 4
partials per batch and adds proj bias.  No device collectives.

Device layouts are feature-on-partition / tokens-on-free ("transposed"):
  xt   [1024, 2048]  x[b]^T                     bf16
  qk   [128 feats, 2048 toks] per feat-block    bf16  (QKV matmul + bias)
  vt   [128 toks, 4, 65] = [v_h | 1]            bf16  (ones col -> softmax sums)
  E^T  [128 Nk, 2*512] = exp(S^T * scale)       bf16  (ACT exp, psum->sbuf)
  AV   psum [65, 512]; row 64 = sums            fp32 accum
  out  [1024, 2048] partial (P_c @ O)^T         fp32

Schedule: the ACT engine (exp, 128 instrs x ~1.1us = ~143us/core) is the
floor; PE total (~137us at 2.4GHz) is just under it.  Everything is ordered
so exp starts as early as the DMA allows and never waits:
  - x^T is loaded chunk-0-first, split across the sync+gpsimd queues;
    weights go on the scalar/vector queues in need-order.
  - prologue emits only what unit (0,0)'s first scores need, then unit 0's
    kb loop self-interleaves its remaining scores (lag 2) while carrying
    v-groups and later k-groups as PE filler.
  - units run cc-major ((0,0),(1,0),(0,1),...); each unit's kb loop emits
    the next unit's scores (exp stream stays dense) plus paced fillers
    (q-groups, then proj groups for completed chunks).
  - softmax normalization: DVE reciprocal on the AV psum sum-row, gpsimd
    partition-broadcast (no DRAM round-trip), DVE multiply into ot.
"""

import numpy as np

B, N, DIM, H, DH = 2, 2048, 1024, 16, 64
SCALE = DH ** -0.5
NCORE = 8
HPC = 4            # heads per core
F = HPC * DH       # 256 features per core-headgroup
CH = 512           # token chunk (matmul moving free dim)
NCH = N // CH      # 4
KT = DIM // 128    # 8 k-tiles over model dim
TB = N // 128      # 16 token blocks
_cache = {}


def _build(debug=False):
    from contextlib import ExitStack

    import concourse.mybir as mybir
    from concourse import bacc
    from concourse.tile import TileContext

    f32 = mybir.dt.float32
    bf16 = mybir.dt.bfloat16
    nc = bacc.Bacc("TRN2", target_bir_lowering=False)

    xt_d = nc.declare_dram_parameter("xt", [DIM, N], bf16, isOutput=False)
    wqk_d = nc.declare_dram_parameter("wqk", [DIM, 2 * F], bf16, isOutput=False)
    wv_d = nc.declare_dram_parameter("wv", [DIM, F], bf16, isOutput=False)
    bqk_d = nc.declare_dram_parameter("bqk", [2 * F], f32, isOutput=False)
    bv_d = nc.declare_dram_parameter("bv", [F], f32, isOutput=False)
    pw_d = nc.declare_dram_parameter("pw", [F, DIM], bf16, isOutput=False)
    out_d = nc.declare_dram_parameter("out", [DIM, N], f32, isOutput=True)
    rscr = nc.dram_tensor("rscr", [2, NCH, 2 * CH], f32)
    if debug:
        dqk = nc.declare_dram_parameter("dqk", [4, NCH, 128, CH], bf16, isOutput=True)
        dvt = nc.declare_dram_parameter("dvt", [TB, 128, HPC, DH + 1], bf16, isOutput=True)
        dot = nc.declare_dram_parameter("dot", [2, NCH, 128, CH], bf16, isOutput=True)
        drec = nc.declare_dram_parameter("drec", [2, NCH, 64, 2 * CH], f32, isOutput=True)
        de = nc.declare_dram_parameter("de", [4, 128, 2 * CH], bf16, isOutput=True)

    xt_r = xt_d.ap().rearrange("(t p) n -> p t n", p=128)     # [128, 8, 2048]
    wqk_r = wqk_d.ap().rearrange("(t p) m -> p t m", p=128)   # [128, 8, 512]
    wv_r = wv_d.ap().rearrange("(t p) m -> p t m", p=128)     # [128, 8, 256]
    pw_r = pw_d.ap().rearrange("(t p) m -> p t m", p=128)     # [128, 2, 1024]
    out_r = out_d.ap().rearrange("(t p) n -> t p n", p=128)   # [8][128, 2048]

    with TileContext(nc) as tc, ExitStack() as st:
        consts = st.enter_context(tc.tile_pool(name="consts", bufs=1))
        qkp = st.enter_context(tc.tile_pool(name="qkp", bufs=1))
        vtp = st.enter_context(tc.tile_pool(name="vtp", bufs=1))
        otp = st.enter_context(tc.tile_pool(name="otp", bufs=1))
        ep = st.enter_context(tc.tile_pool(name="ep", bufs=2))
        recp = st.enter_context(tc.tile_pool(name="recp", bufs=2))
        outs = st.enter_context(tc.tile_pool(name="outs", bufs=3))
        ps_s = st.enter_context(tc.tile_pool(name="ps_s", bufs=2, space="PSUM"))
        ps_mm = st.enter_context(tc.tile_pool(name="ps_mm", bufs=2, space="PSUM"))
        ps_av = st.enter_context(tc.tile_pool(name="ps_av", bufs=2, space="PSUM"))

        # ---- DMA: weights on scalar queue (need-order, 128-feat blocks), x
        # chunk low-halves on sync, high-halves + wv + pw on gpsimd.
        bqk_sb = consts.tile([128, 4], f32)
        nc.scalar.dma_start(out=bqk_sb, in_=bqk_d.ap().rearrange("(f p) -> p f", p=128))
        bv_sb = consts.tile([128, F], f32)
        nc.scalar.dma_start(out=bv_sb, in_=bv_d.ap().partition_broadcast(128))
        # 2D per-ktile weight DMAs, k-half first (unit (0,0)'s scores need k
        # before q); few scalar-queue issues so the exps behind them start early.
        wqk_sb = consts.tile([128, KT, 2 * F], bf16, tag="wqk", name="wqk")
        for t in range(KT):
            nc.scalar.dma_start(out=wqk_sb[:, t, F:], in_=wqk_r[:, t, F:])
        for t in range(KT):
            nc.scalar.dma_start(out=wqk_sb[:, t, :F], in_=wqk_r[:, t, :F])
        wv_sb = consts.tile([128, KT, F], bf16, tag="wv", name="wv")
        pw_sb = consts.tile([128, 2, DIM], bf16, tag="pw", name="pw")
        for t in range(KT):
            nc.gpsimd.dma_start(out=wv_sb[:, t, :], in_=wv_r[:, t, :])
        for t in range(2):
            nc.gpsimd.dma_start(out=pw_sb[:, t, :], in_=pw_r[:, t, :])

        # x chunk-major 2D loads on the sync queue (HWDGE)
        xc_sb = [consts.tile([128, KT, CH], bf16, tag=f"x{ch}", name=f"x{ch}")
                 for ch in range(NCH)]
        for ch in range(NCH):
            cs = slice(ch * CH, (ch + 1) * CH)
            for t in range(KT):
                nc.sync.dma_start(out=xc_sb[ch][:, t, :], in_=xt_r[:, t, cs])

        # ---- SBUF result tiles
        # qk_sb[i][ch]: i=0,1 -> q features of head pair i; i=2,3 -> k of pair i-2
        qk_sb = [[qkp.tile([128, CH], bf16, tag=f"qk{i}_{ch}", name=f"qk{i}_{ch}")
                  for ch in range(NCH)] for i in range(4)]
        vt_sb = [vtp.tile([128, HPC, DH + 1], bf16, tag=f"vt{tb}", name=f"vt{tb}")
                 for tb in range(TB)]
        ot_sb = [[otp.tile([128, CH], bf16, tag=f"ot{t}_{cc}", name=f"ot{t}_{cc}")
                  for cc in range(NCH)] for t in range(2)]

        # ---- emitters ------------------------------------------------------
        qk_ps = {}

        def emit_qk_half(i, ch, half):
            """Half (4 ktiles) of a 128-feature QKV block; bias-add on half 1."""
            wo = i * 128
            if half == 0:
                qk_ps[(i, ch)] = ps_mm.tile([128, CH], f32, tag="mm", name=f"qk{i}_{ch}ps")
            ps = qk_ps[(i, ch)]
            for t in range(half * 4, half * 4 + 4):
                nc.tensor.matmul(
                    ps, wqk_sb[:, t, wo:wo + 128], xc_sb[ch][:, t, :],
                    start=(t == 0), stop=(t == KT - 1),
                )
            if half == 1:
                nc.vector.tensor_scalar_add(
                    out=qk_sb[i][ch], in0=ps, scalar1=bqk_sb[:, i:i + 1],
                )
                del qk_ps[(i, ch)]
                if debug:
                    nc.sync.dma_start(out=dqk.ap()[i, ch], in_=qk_sb[i][ch])

        def emit_qk(i, ch):
            emit_qk_half(i, ch, 0)
            emit_qk_half(i, ch, 1)

        def emit_v(tb):
            ps = ps_mm.tile([128, F], f32, tag="mm", name=f"v{tb}ps")
            for t in range(KT):
                nc.tensor.matmul(
                    ps,
                    xc_sb[tb // 4][:, t, (tb % 4) * 128:(tb % 4 + 1) * 128],
                    wv_sb[:, t, :],
                    start=(t == 0), stop=(t == KT - 1),
                )
            for hh in range(HPC):
                nc.vector.tensor_add(
                    out=vt_sb[tb][:, hh, :DH],
                    in0=ps[:, hh * DH:(hh + 1) * DH],
                    in1=bv_sb[:, hh * DH:(hh + 1) * DH],
                )
            nc.vector.memset(vt_sb[tb][:, :, DH:], 1.0)
            if debug:
                nc.sync.dma_start(out=dvt.ap()[tb], in_=vt_sb[tb])

        et_store = {}

        def emit_s(u, kb):
            """Scores + exp for unit u=(hp,cc), k-block kb: S^T [128k, 2*512q]."""
            hp, cc = u
            sp = ps_s.tile([128, 2 * CH], f32, tag="sp", name=f"sp{hp}_{cc}_{kb}")
            for j in range(2):
                kt = qk_sb[2 + hp][kb // 4]
                qt = qk_sb[hp][cc]
                nc.tensor.matmul(
                    sp[:, j * CH:(j + 1) * CH],
                    kt[j * 64:j * 64 + 64, (kb % 4) * 128:(kb % 4 + 1) * 128],
                    qt[j * 64:j * 64 + 64, :],
                    start=True, stop=True,
                )
            e = ep.tile([128, 2 * CH], bf16, tag=f"e{kb}", name=f"e{hp}_{cc}_{kb}")
            nc.scalar.activation(
                out=e, in_=sp, func=mybir.ActivationFunctionType.Exp, scale=SCALE,
            )
            et_store[(u, kb)] = e
            if debug and u == (0, 0) and kb < 4:
                nc.sync.dma_start(out=de.ap()[kb], in_=e)

        def emit_av(u, avs, kb):
            hp, cc = u
            e = et_store[(u, kb)] if kb < TB - 1 else et_store.pop((u, kb))
            if kb == TB - 1:
                for k2 in range(TB - 1):
                    et_store.pop((u, k2))
            for j in range(2):
                nc.tensor.matmul(
                    avs[j],
                    vt_sb[kb][:, 2 * hp + j, :],
                    e[:, j * CH:(j + 1) * CH],
                    start=(kb == 0), stop=(kb == TB - 1),
                )

        ones_sb = consts.tile([1, 64], f32, tag="ones", name="ones")
        nc.vector.memset(ones_sb, 1.0)

        def finish_unit(u, avs, last=False):
            """Softmax normalize: 1/sumexp (DVE), broadcast, mul -> ot."""
            hp, cc = u
            # Copy avs out of PSUM promptly (frees the ps_av ring for the next
            # unit); custom-DVE ops also mishandle PSUM partition offsets on
            # HW, so reciprocal reads the SBUF copy.
            stg = recp.tile([64, 2 * CH], f32, tag="stg", name=f"stg{hp}_{cc}")
            sums = recp.tile([1, 2 * CH], f32, tag="sums", name=f"sums{hp}_{cc}")
            for j in range(2):
                nc.vector.tensor_copy(
                    out=stg[:, j * CH:(j + 1) * CH], in_=avs[j][0:64, :]
                )
                nc.vector.tensor_copy(
                    out=sums[:, j * CH:(j + 1) * CH], in_=avs[j][64:65, :]
                )
            rec = recp.tile([1, 2 * CH], f32, tag="rec", name=f"rec{hp}_{cc}")
            nc.vector.reciprocal_approx_fast(out=rec, in_=sums)
            if last:
                # scores are done: broadcast via a PE ones-matmul into the free
                # ps_s ring -- avoids the ~7us DRAM round-trip on the tail.
                rec64 = ps_s.tile([64, 2 * CH], f32, tag="sp", name=f"rb{hp}_{cc}")
                for j in range(2):
                    nc.tensor.matmul(
                        rec64[:, j * CH:(j + 1) * CH], ones_sb,
                        rec[:, j * CH:(j + 1) * CH], start=True, stop=True,
                    )
            else:
                nc.gpsimd.dma_start(out=rscr.ap()[hp, cc], in_=rec)
                rec64 = recp.tile([64, 2 * CH], f32, tag="rec64", name=f"rb{hp}_{cc}")
                nc.gpsimd.dma_start(
                    out=rec64, in_=rscr.ap()[hp, cc].partition_broadcast(64)
                )
            if debug:
                nc.sync.dma_start(out=drec.ap()[hp, cc], in_=rec64)
            for j in range(2):
                # gpsimd, not DVE: these wait on the rec64 round-trip, and the
                # in-order DVE queue must stay clear for os copies.  (last
                # unit: rec64 is PSUM, which gpsimd can't read -- use DVE; its
                # wait is only on the quick PE broadcast.)
                (nc.vector if last else nc.gpsimd).tensor_mul(
                    out=ot_sb[hp][cc][j * 64:j * 64 + 64, :],
                    in0=stg[:, j * CH:(j + 1) * CH],
                    in1=rec64[:, j * CH:(j + 1) * CH],
                )
            if debug:
                nc.sync.dma_start(out=dot.ap()[hp, cc], in_=ot_sb[hp][cc])

        def emit_proj(fb, cc, ring="mm", ceng=None):
            pool = {"mm": ps_mm, "sp": ps_s}[ring]
            ps = pool.tile([128, CH], f32, tag=ring, name=f"pj{fb}_{cc}")
            for t in range(2):
                nc.tensor.matmul(
                    ps, pw_sb[:, t, fb * 128:(fb + 1) * 128], ot_sb[t][cc],
                    start=(t == 0), stop=(t == 1),
                )
            os = outs.tile([128, CH], f32, tag="os", name=f"os{fb}_{cc}")
            if ceng is nc.scalar:
                nc.scalar.copy(out=os, in_=ps)
            else:
                (ceng or nc.vector).tensor_copy(out=os, in_=ps)
            nc.sync.dma_start(out=out_r[fb][:, cc * CH:(cc + 1) * CH], in_=os)

        # ---- schedule ------------------------------------------------------
        units = [(hp, cc) for cc in range(NCH) for hp in (0, 1)]  # cc-major
        u0, u1 = units[0], units[1]

        # Prologue: ALL of units 0 and 1's scores (the ACT stream for the first
        # ~35us), deadline-interleaved with the k/q/v groups they and unit 0's
        # AV need.  s-emissions are spaced ~1.1us apart in PE terms so the
        # ps_s ring (depth 2) never stalls the PE queue.
        A = lambda kb: emit_s(u0, kb)
        Bs = lambda kb: emit_s(u1, kb)
        emit_qk(2, 0)
        emit_qk(0, 0)
        A(0); A(1)
        emit_qk(3, 0)
        A(2)
        emit_qk(1, 0)
        A(3)
        emit_v(0); Bs(0)
        emit_v(1); Bs(1)
        emit_qk_half(2, 1, 0); Bs(2)
        emit_qk_half(2, 1, 1); Bs(3)
        emit_v(2); A(4)
        emit_qk_half(3, 1, 0); A(5)
        emit_qk_half(3, 1, 1); Bs(4)
        emit_v(3); A(6)
        emit_v(4); Bs(5)
        emit_qk_half(2, 2, 0); A(7)
        emit_qk_half(2, 2, 1); Bs(6)
        emit_v(5); Bs(7)
        emit_qk_half(3, 2, 0); A(8)
        emit_qk_half(3, 2, 1); A(9)
        emit_v(6); Bs(8)
        emit_v(7); A(10)
        emit_qk_half(2, 3, 0); Bs(9)
        emit_qk_half(2, 3, 1); A(11)
        emit_v(8); Bs(10)
        emit_qk_half(3, 3, 0); A(12)
        emit_qk_half(3, 3, 1); Bs(11)
        emit_v(9); A(13)
        emit_v(10); Bs(12)
        emit_qk_half(0, 1, 0); A(14)
        emit_qk_half(0, 1, 1); Bs(13)
        emit_v(11); A(15)
        Bs(14); Bs(15)

        # Unit i's loop emits unit i+2's scores (i+1's ran one loop earlier),
        # so the exp stream stays dense through the whole run; fillers:
        #   v(12..15) early in unit 0; q-group for unit j+3 late in unit j;
        #   proj(cc) spread over units 2cc+2 and 2cc+3.
        flr = {}
        for kb, tb in enumerate(range(12, 16)):
            flr[(0, kb)] = [lambda tb=tb: emit_v(tb)]
        qsched = {0: (1, 1), 1: (0, 2), 2: (1, 2), 3: (0, 3), 4: (1, 3)}
        for j, (qi, qc) in qsched.items():
            flr.setdefault((j, 13), []).append(
                lambda qi=qi, qc=qc: emit_qk_half(qi, qc, 0))
            flr.setdefault((j, 15), []).append(
                lambda qi=qi, qc=qc: emit_qk_half(qi, qc, 1))
        for cc in range(3):
            for i in (2 * cc + 2, 2 * cc + 3):
                for s_, fb in zip((2, 6, 10, 14), range(4)):
                    flr.setdefault((i, s_), []).append(
                        lambda fb=fb + 4 * (i - 2 * cc - 2), cc=cc:
                        emit_proj(fb, cc))

        for i, u in enumerate(units):
            hp, cc = u
            avs = [
                ps_av.tile([65, CH], f32, tag="av", name=f"av{hp}_{cc}_{j}")
                for j in range(2)
            ]
            nn = units[i + 2] if i + 2 < len(units) else None
            for kb in range(TB):
                for fn in flr.get((i, kb), ()):
                    fn()
                if nn is not None:
                    emit_s(nn, kb)
                emit_av(u, avs, kb)
            finish_unit(u, avs, last=(i == len(units) - 1))

        # tail: chunk 3's projection, alternating PSUM rings and copy engines
        # so the 8 groups pipeline instead of serializing on one ring
        for fb in range(KT):
            emit_proj(fb, 3, ring="mm" if fb % 2 == 0 else "sp",
                      ceng=nc.scalar if fb % 2 == 0 else nc.vector)

    nc.finalize()
    return nc


def _in_maps(x, qkv_w, qkv_b, proj_w):
    import ml_dtypes

    bf = ml_dtypes.bfloat16
    maps = []
    for c in range(NCORE):
        b, hg = c // 4, c % 4
        fs = slice(hg * F, (hg + 1) * F)
        wqk = np.concatenate([qkv_w[fs], qkv_w[DIM:][fs]], 0)        # [512,1024]
        bqk = np.concatenate([qkv_b[fs], qkv_b[DIM:][fs]], 0)
        maps.append({
            "xt": np.ascontiguousarray(x[b].T).astype(bf),
            "wqk": np.ascontiguousarray(wqk.T).astype(bf),
            "wv": np.ascontiguousarray(qkv_w[2 * DIM:][fs].T).astype(bf),
            "bqk": np.ascontiguousarray(bqk),
            "bv": np.ascontiguousarray(qkv_b[2 * DIM:][fs]),
            "pw": np.ascontiguousarray(proj_w[:, fs].T).astype(bf),
        })
    return maps


def _run(inputs, trace=False, trace_kwargs=None):
    from concourse.bass_utils import run_bass_kernel_spmd

    if "nc" not in _cache:
        _cache["nc"] = _build()
    nc = _cache["nc"]
    maps = _in_maps(inputs["x"], inputs["qkv_w"], inputs["qkv_b"], inputs["proj_w"])
    res = run_bass_kernel_spmd(
        nc, maps, list(range(NCORE)), trace=trace, **(trace_kwargs or {})
    )
    outs = [r["out"] for r in res.results]              # [1024, 2048] partials
    full = np.empty((B, N, DIM), dtype=np.float32)
    for b in range(B):
        acc = np.array(outs[4 * b], dtype=np.float32)
        for c in range(4 * b + 1, 4 * b + 4):
            acc += np.asarray(outs[c], dtype=np.float32)
        full[b] = acc.T + inputs["proj_b"]
    return full, res


def kernel(**inputs) -> np.ndarray:
    out, _ = _run(inputs, trace=False)
    return out
